# revision 7
# baseline (speedup 1.0000x reference)
"""Block-diagonal dense (nn_BlockDiagonalDense) Trainium2 Bass kernel.

Math: x [B=4, T=4096, F=4096] fp32; per token, features are grouped into
512 blocks of 8; each block is multiplied by its own 8x8 matrix
(kernel [16 heads, 32 blocks, 8, 8]) and bias added (bias is zeros in
setup_inputs, but we fold it in anyway).

Strategy:
  - Data-parallel over tokens across 8 cores (16384 tokens -> 2048/core).
  - Weights are expanded host-side into 32 chunks of 128x128 block-diagonal
    matrices (one per 128 consecutive features), replicated to every core.
  - On-chip per 128-token tile: PE transpose of each 128-feature chunk
    (fp32, via identity matmul) -> PSUM -> copy to SBUF (ScalarE) ->
    PE matmul lhsT=x^T chunk (stationary), rhs=W chunk (moving) giving
    token-major output in PSUM -> VectorE drain with fused bias add ->
    contiguous DMA out.
"""

import sys

if "/opt/trn_rl_repo" not in sys.path:
    sys.path.insert(0, "/opt/trn_rl_repo")

import numpy as np

NUM_HEADS = 16
BLOCK_SIZE = 8
FEATURES = 4096
HEAD_DIM = FEATURES // NUM_HEADS  # 256
BLOCK_DIM = HEAD_DIM // BLOCK_SIZE  # 32

N_CORES = 8
TOKENS_TOTAL = 4 * 4096  # 16384
TOK_PER_CORE = TOKENS_TOTAL // N_CORES  # 2048

P = 128  # partitions
N_CHUNKS = FEATURES // P  # 32 chunks of 128 features
CG = 4  # chunks per group (512 output cols per PSUM bank)

_NC_CACHE = {}


def build_nc(tok_per_core=TOK_PER_CORE, repeats=1):
    """Build the Bass program for one core processing [tok_per_core, 4096].

    repeats>1 wraps the whole body in a hardware loop doing identical work
    (same inputs, same outputs) -- used only for slope-based device timing.
    """
    import contextlib

    import concourse.bass as bass
    import concourse.mybir as mybir
    from concourse import bacc
    from concourse.masks import make_identity
    from concourse.tile import TileContext

    f32 = mybir.dt.float32
    nc = bacc.Bacc(None, target_bir_lowering=False)

    x = nc.declare_dram_parameter("x", [tok_per_core, FEATURES], f32, isOutput=False)
    # w: [128 (fi within chunk), 32*128 (chunk-major, fo within chunk)]
    w = nc.declare_dram_parameter("w", [P, N_CHUNKS * P], f32, isOutput=False)
    b = nc.declare_dram_parameter("b", [FEATURES], f32, isOutput=False)
    y = nc.declare_dram_parameter("y", [tok_per_core, FEATURES], f32, isOutput=True)

    n_tiles = tok_per_core // P

    with TileContext(nc) as tc:
        with (
            tc.tile_pool(name="const", bufs=1) as const_pool,
            tc.tile_pool(name="xin", bufs=3) as x_pool,
            tc.tile_pool(name="yout", bufs=3) as y_pool,
            tc.tile_pool(name="xt", bufs=4) as xt_pool,
            tc.tile_pool(name="pst", bufs=3, space="PSUM") as pst_pool,
            tc.tile_pool(name="psy", bufs=3, space="PSUM") as psy_pool,
        ):
            w_sb = const_pool.tile([P, N_CHUNKS * P], f32)
            nc.sync.dma_start(out=w_sb, in_=w[:, :])

            # bias replicated across all 128 partitions (partition-stride 0)
            bias_sb = const_pool.tile([P, FEATURES], f32)
            b_ap = b[:]
            bias_bcast = bass.AP(
                tensor=b_ap.tensor, offset=b_ap.offset, ap=[[0, P], [1, FEATURES]]
            )
            nc.gpsimd.dma_start(out=bias_sb, in_=bias_bcast)

            ident = const_pool.tile([P, P], f32)
            make_identity(nc, ident)

            rep_ctx = (
                tc.For_i(0, repeats, 1) if repeats > 1 else contextlib.nullcontext()
            )
            with rep_ctx:
                for ti in range(n_tiles):
                    x_tile = x_pool.tile([P, FEATURES], f32)
                    nc.sync.dma_start(out=x_tile, in_=x[ti * P : (ti + 1) * P, :])

                    y_tile = y_pool.tile([P, FEATURES], f32)

                    for g in range(N_CHUNKS // CG):
                        ps_t = pst_pool.tile([P, CG * P], f32)
                        for k in range(CG):
                            c = g * CG + k
                            nc.tensor.transpose(
                                ps_t[:, k * P : (k + 1) * P],
                                x_tile[:, c * P : (c + 1) * P],
                                ident,
                            )
                        xt = xt_pool.tile([P, CG * P], f32)
                        nc.scalar.copy(xt, ps_t)

                        ps_y = psy_pool.tile([P, CG * P], f32)
                        for k in range(CG):
                            c = g * CG + k
                            nc.tensor.matmul(
                                ps_y[:, k * P : (k + 1) * P],
                                xt[:, k * P : (k + 1) * P],
                                w_sb[:, c * P : (c + 1) * P],
                            )
                        # drain + fused bias add (bias varies along free dim)
                        nc.vector.tensor_add(
                            y_tile[:, g * CG * P : (g + 1) * CG * P],
                            ps_y,
                            bias_sb[:, g * CG * P : (g + 1) * CG * P],
                        )

                    # out-DMA on the ACT HWDGE ring so it overlaps the
                    # SP-ring input DMAs
                    nc.scalar.dma_start(out=y[ti * P : (ti + 1) * P, :], in_=y_tile)

    nc.finalize()
    return nc


def expand_weights(kern):
    """kernel [16, 32, 8, 8] -> [128, 32*128] chunk-major block-diagonal."""
    kern = np.asarray(kern, dtype=np.float32)
    wd = np.zeros((N_CHUNKS, P, P), dtype=np.float32)
    for c in range(N_CHUNKS):
        h = c // 2
        for j in range(16):
            bd = 16 * (c % 2) + j
            wd[c, 8 * j : 8 * j + 8, 8 * j : 8 * j + 8] = kern[h, bd]
    # [chunk, fi, fo] -> [fi, chunk*128 + fo]
    return np.ascontiguousarray(wd.transpose(1, 0, 2).reshape(P, N_CHUNKS * P))


def reference_numpy(x, kern, bias):
    xb = np.asarray(x, np.float32).reshape(-1, NUM_HEADS, BLOCK_DIM, BLOCK_SIZE)
    k = np.asarray(kern, np.float32)
    y = np.einsum("nhbs,hbst->nhbt", xb, k) + np.asarray(bias, np.float32)
    return y.reshape(x.shape)


_LAST_EXEC_NS = None


def kernel(**inputs):
    """Full inputs in, full output out. Shards tokens across 8 cores."""
    global _LAST_EXEC_NS
    import os

    from concourse.bass_utils import run_bass_kernel_spmd

    x = np.ascontiguousarray(np.asarray(inputs["x"], dtype=np.float32))
    kern = np.asarray(inputs["kernel"], dtype=np.float32)
    bias = np.ascontiguousarray(
        np.asarray(inputs["bias"], dtype=np.float32).reshape(FEATURES)
    )

    orig_shape = x.shape
    xf = x.reshape(TOKENS_TOTAL, FEATURES)
    w = expand_weights(kern)

    if "nc" not in _NC_CACHE:
        _NC_CACHE["nc"] = build_nc()
    nc = _NC_CACHE["nc"]

    in_maps = [
        {
            "x": xf[c * TOK_PER_CORE : (c + 1) * TOK_PER_CORE],
            "w": w,
            "b": bias,
        }
        for c in range(N_CORES)
    ]

    trace = bool(os.environ.get("BASS_KERNEL_TRACE"))
    res = run_bass_kernel_spmd(nc, in_maps, list(range(N_CORES)), trace=trace)
    _LAST_EXEC_NS = res.exec_time_ns

    y = np.concatenate([r["y"] for r in res.results], axis=0)
    return y.reshape(orig_shape)
